# revision 2
# baseline (speedup 1.0000x reference)
# Trainium2 Bass kernel for CausalTemporalAttention
#   y = LayerNorm(x + Wo(softmax(QK^T/sqrt(hd) + causal + rel_bias) V))
#
# Strategy (8 NeuronCores):
#   Phase 1 (tensor-parallel over heads, 2 heads/core):
#     Each core computes Q^T,K^T (transposed via host-pretransposed x^T),
#     V (natural layout with an appended ones-column so the PV matmul also
#     produces softmax denominators), then causal attention with the
#     rel_bias streamed in transposed bf16 form with the causal mask baked
#     in on the host as -60000 (exp -> exact 0).  Softmax is computed
#     without max-subtraction (scores are bounded ~|4| for this problem).
#     Output: normalized attended^T (128 channels x 4096 rows) per core.
#   Host: concatenates the 8 channel-blocks (pure gather/reshard).
#   Phase 2 (sequence-parallel, 512 rows/core):
#     out-projection + residual + LayerNorm on each core's row block.
#     bv/bo are folded into a single bias vector on the host
#     (b2 = bo + bv @ Wo.T, parameter algebra only).
#
# All matmul math runs in bf16 with f32 PSUM accumulation (validated:
# absmax err ~5e-3 = 1e-3 of output scale vs the f32 reference).

import os
from contextlib import ExitStack

import ml_dtypes
import numpy as np

B, S, D, H, HD = 2, 2048, 1024, 16, 64
N = B * S            # 4096 flattened rows
NCORES = 8
HPC = H // NCORES    # 2 heads per core
ROWS = N // NCORES   # 512 rows per core in phase 2
QC = 512             # query-chunk width (phase 1)
KT = 128             # key-tile height (phase 1)
NEG = -60000.0       # causal mask additive value (exp -> 0 exactly in f32)
EPS = 1e-5

BF = ml_dtypes.bfloat16

_CACHE: dict = {}
LAST_PROFILE: list = []   # [(phase, exec_time_ns)], filled when BASS_TRACE=1


def _mybir():
    from concourse import mybir
    return mybir


def _build_phase1():
    import concourse.tile as tile
    from concourse import bacc
    mybir = _mybir()
    bf, f32 = mybir.dt.bfloat16, mybir.dt.float32
    Ident = mybir.ActivationFunctionType.Identity
    Exp = mybir.ActivationFunctionType.Exp

    nc = bacc.Bacc(
        "TRN2",
        target_bir_lowering=False,
        debug=False,
        enable_asserts=False,
        num_devices=NCORES,
    )
    xT = nc.dram_tensor("xT", (D, N), bf, kind="ExternalInput").ap()
    wqT = nc.dram_tensor("wqT", (D, 128), bf, kind="ExternalInput").ap()
    wkT = nc.dram_tensor("wkT", (D, 128), bf, kind="ExternalInput").ap()
    wvT = nc.dram_tensor("wvT", (D, 128), bf, kind="ExternalInput").ap()
    bq = nc.dram_tensor("bq", (128, 1), f32, kind="ExternalInput").ap()
    bk = nc.dram_tensor("bk", (128, 1), f32, kind="ExternalInput").ap()
    relT = nc.dram_tensor("relT", (HPC, S, S), bf, kind="ExternalInput").ap()
    attT = nc.dram_tensor("attT", (128, N), bf, kind="ExternalOutput").ap()

    with tile.TileContext(nc) as tc, ExitStack() as ctx:
        singles = ctx.enter_context(tc.tile_pool(name="singles", bufs=1))
        relp = ctx.enter_context(tc.tile_pool(name="relp", bufs=2))
        stgp = ctx.enter_context(tc.tile_pool(name="stgp", bufs=1))
        ptp = ctx.enter_context(tc.tile_pool(name="ptp", bufs=2))
        smallp = ctx.enter_context(tc.tile_pool(name="smallp", bufs=2))
        attp = ctx.enter_context(tc.tile_pool(name="attp", bufs=2))
        psA = ctx.enter_context(tc.tile_pool(name="psA", bufs=4, space="PSUM"))
        psPV = ctx.enter_context(tc.tile_pool(name="psPV", bufs=2, space="PSUM"))
        psBC = ctx.enter_context(tc.tile_pool(name="psBC", bufs=2, space="PSUM"))

        # ---- load inputs ----
        xt = singles.tile([128, 8, N], bf)
        xTr = xT.rearrange("(c p) n -> c p n", p=128)
        for i in range(8):
            nc.sync.dma_start(out=xt[:, i, :], in_=xTr[i])

        wq = singles.tile([128, 8, 128], bf)
        wk = singles.tile([128, 8, 128], bf)
        wv = singles.tile([128, 8, 128], bf)
        for w_sb, w_d in ((wq, wqT), (wk, wkT), (wv, wvT)):
            w_r = w_d.rearrange("(c p) m -> c p m", p=128)
            for i in range(8):
                nc.sync.dma_start(out=w_sb[:, i, :], in_=w_r[i])
        bq_sb = singles.tile([128, 1], f32)
        nc.sync.dma_start(out=bq_sb, in_=bq)
        bk_sb = singles.tile([128, 1], f32)
        nc.sync.dma_start(out=bk_sb, in_=bk)
        ones_sb = singles.tile([1, 64], f32)
        nc.vector.memset(ones_sb, 1.0)

        qt = singles.tile([128, N], bf)    # Q^T (2 heads stacked), pre-scaled
        ktl = singles.tile([128, N], bf)   # K^T
        vt = singles.tile([128, 32, 130], bf)  # V natural per k-tile; per head 64 ch + ones col
        nc.vector.memset(vt[:, :, 64:65], 1.0)
        nc.vector.memset(vt[:, :, 129:130], 1.0)

        # ---- Q/K projections (transposed layout) ----
        for w_sb, b_sb, dst in ((wq, bq_sb, qt), (wk, bk_sb, ktl)):
            for ncol in range(N // QC):
                p = psA.tile([128, QC], f32, tag="ps")
                for i in range(8):
                    nc.tensor.matmul(
                        p,
                        lhsT=w_sb[:, i, :],
                        rhs=xt[:, i, ncol * QC:(ncol + 1) * QC],
                        start=(i == 0),
                        stop=(i == 7),
                    )
                nc.scalar.activation(
                    out=dst[:, ncol * QC:(ncol + 1) * QC],
                    in_=p,
                    func=Ident,
                    bias=b_sb[:, 0:1],
                    scale=1.0,
                )

        # ---- V projection (natural layout) ----
        for m in range(N // KT):
            p = psA.tile([128, 128], f32, tag="ps")
            for i in range(8):
                nc.tensor.matmul(
                    p,
                    lhsT=xt[:, i, m * KT:(m + 1) * KT],
                    rhs=wv[:, i, :],
                    start=(i == 0),
                    stop=(i == 7),
                )
            nc.vector.tensor_copy(out=vt[:, m, 0:64], in_=p[:, 0:64])
            nc.vector.tensor_copy(out=vt[:, m, 65:129], in_=p[:, 64:128])

        # ---- attention ----
        relr = relT.rearrange("h (t p) q -> h p t q", p=128)
        nqc = S // QC             # 4 query chunks per batch
        for h in range(HPC):
            for qc in range(nqc):
                nkt = (qc + 1) * (QC // KT)   # causal: k-tiles needed
                rel = relp.tile([128, S // KT, QC], bf, tag="rel")
                nc.sync.dma_start(
                    out=rel[:, 0:nkt, :],
                    in_=relr[h][:, 0:nkt, qc * QC:(qc + 1) * QC],
                )
                for b in range(B):
                    qoff = b * S + qc * QC
                    stg = stgp.tile([128, S // KT, QC], f32, tag="stg")
                    pts = ptp.tile([128, S // KT, QC], bf, tag="pt")
                    for kt in range(nkt):
                        sp = psA.tile([128, QC], f32, tag="ps")
                        nc.tensor.matmul(
                            sp,
                            lhsT=ktl[64 * h:64 * (h + 1), b * S + kt * KT: b * S + (kt + 1) * KT],
                            rhs=qt[64 * h:64 * (h + 1), qoff:qoff + QC],
                            start=True,
                            stop=True,
                        )
                        nc.vector.tensor_add(out=stg[:, kt, :], in0=sp, in1=rel[:, kt, :])
                    nc.scalar.activation(out=pts[:, 0:nkt, :], in_=stg[:, 0:nkt, :], func=Exp)
                    pv = psPV.tile([128, QC], f32, tag="pv")
                    for kt in range(nkt):
                        nc.tensor.matmul(
                            pv[0:65, :],
                            lhsT=vt[:, b * (S // KT) + kt, h * 65:(h + 1) * 65],
                            rhs=pts[:, kt, :],
                            start=(kt == 0),
                            stop=(kt == nkt - 1),
                        )
                    rec = smallp.tile([1, QC], f32, tag="rec")
                    nc.vector.reciprocal(out=rec, in_=pv[64:65, :])
                    bc = psBC.tile([64, QC], f32, tag="bc")
                    nc.tensor.matmul(bc, lhsT=ones_sb, rhs=rec, start=True, stop=True)
                    bcs = smallp.tile([64, QC], f32, tag="bcs")
                    nc.scalar.copy(out=bcs, in_=bc)
                    att = attp.tile([64, QC], bf, tag="att")
                    nc.vector.tensor_mul(out=att, in0=pv[0:64, :], in1=bcs)
                    nc.sync.dma_start(
                        out=attT[64 * h:64 * (h + 1), qoff:qoff + QC], in_=att
                    )
    nc.compile()
    return nc


def _build_phase2():
    import concourse.tile as tile
    from concourse import bacc
    mybir = _mybir()
    bf, f32 = mybir.dt.bfloat16, mybir.dt.float32
    Sqrt = mybir.ActivationFunctionType.Sqrt

    nc = bacc.Bacc(
        "TRN2",
        target_bir_lowering=False,
        debug=False,
        enable_asserts=False,
        num_devices=NCORES,
    )
    aT = nc.dram_tensor("aT", (D, ROWS), bf, kind="ExternalInput").ap()
    woT = nc.dram_tensor("woT", (D, D), bf, kind="ExternalInput").ap()
    xr = nc.dram_tensor("xr", (ROWS, D), f32, kind="ExternalInput").ap()
    b2 = nc.dram_tensor("b2", (1, D), f32, kind="ExternalInput").ap()
    gam = nc.dram_tensor("gam", (1, D), f32, kind="ExternalInput").ap()
    bet = nc.dram_tensor("bet", (1, D), f32, kind="ExternalInput").ap()
    yout = nc.dram_tensor("yout", (ROWS, D), f32, kind="ExternalOutput").ap()

    import concourse.bass as bass

    def bcast(ap_1d):
        # (1, D) dram -> [0-stride over 128 partitions, D] for broadcast DMA
        return bass.AP(tensor=ap_1d.tensor, offset=ap_1d.offset,
                       ap=[[0, 128]] + list(ap_1d.ap[1:]))

    with tile.TileContext(nc) as tc, ExitStack() as ctx:
        singles = ctx.enter_context(tc.tile_pool(name="singles", bufs=1))
        work = ctx.enter_context(tc.tile_pool(name="work", bufs=3))
        ps = ctx.enter_context(tc.tile_pool(name="ps", bufs=4, space="PSUM"))

        at = singles.tile([128, 8, ROWS], bf)
        a_r = aT.rearrange("(c p) n -> c p n", p=128)
        for i in range(8):
            nc.sync.dma_start(out=at[:, i, :], in_=a_r[i])
        wo = singles.tile([128, 8, D], bf)
        w_r = woT.rearrange("(c p) n -> c p n", p=128)
        for i in range(8):
            nc.sync.dma_start(out=wo[:, i, :], in_=w_r[i])
        b2_sb = singles.tile([128, D], f32)
        nc.sync.dma_start(out=b2_sb, in_=bcast(b2))
        g_sb = singles.tile([128, D], f32)
        nc.sync.dma_start(out=g_sb, in_=bcast(gam))
        be_sb = singles.tile([128, D], f32)
        nc.sync.dma_start(out=be_sb, in_=bcast(bet))
        eps_sb = singles.tile([128, 1], f32)
        nc.vector.memset(eps_sb, EPS)

        for m in range(ROWS // 128):
            xr_sb = work.tile([128, D], f32, tag="xr")
            nc.sync.dma_start(out=xr_sb, in_=xr[m * 128:(m + 1) * 128, :])
            y = work.tile([128, D], f32, tag="y")
            for ncc in range(D // 512):
                p = ps.tile([128, 512], f32, tag="p")
                for i in range(8):
                    nc.tensor.matmul(
                        p,
                        lhsT=at[:, i, m * 128:(m + 1) * 128],
                        rhs=wo[:, i, ncc * 512:(ncc + 1) * 512],
                        start=(i == 0),
                        stop=(i == 7),
                    )
                nc.vector.tensor_add(
                    out=y[:, ncc * 512:(ncc + 1) * 512],
                    in0=p,
                    in1=xr_sb[:, ncc * 512:(ncc + 1) * 512],
                )
            nc.vector.tensor_add(out=y, in0=y, in1=b2_sb)
            stats = work.tile([128, 2, 6], f32, tag="st")
            nc.vector.bn_stats(out=stats[:, 0, :], in_=y[:, 0:512])
            nc.vector.bn_stats(out=stats[:, 1, :], in_=y[:, 512:1024])
            mv = work.tile([128, 2], f32, tag="mv")
            nc.vector.bn_aggr(out=mv, in_=stats)
            rstd = work.tile([128, 1], f32, tag="rs")
            nc.scalar.activation(out=rstd, in_=mv[:, 1:2], func=Sqrt,
                                 bias=eps_sb[:, 0:1], scale=1.0)
            nc.vector.reciprocal(out=rstd, in_=rstd)
            nc.vector.tensor_scalar(
                out=y, in0=y,
                scalar1=mv[:, 0:1], scalar2=rstd,
                op0=mybir.AluOpType.subtract, op1=mybir.AluOpType.mult,
            )
            nc.vector.tensor_mul(out=y, in0=y, in1=g_sb)
            nc.vector.tensor_add(out=y, in0=y, in1=be_sb)
            nc.sync.dma_start(out=yout[m * 128:(m + 1) * 128, :], in_=y)
    nc.compile()
    return nc


def _run(nc, in_maps, phase):
    from concourse.bass_utils import run_bass_kernel_spmd
    trace = bool(os.environ.get("BASS_TRACE"))
    res = run_bass_kernel_spmd(nc, in_maps, list(range(NCORES)), trace=trace)
    if trace:
        LAST_PROFILE.append((phase, res.exec_time_ns))
    return res.results


def kernel(**inputs):
    f32 = np.float32
    x = np.asarray(inputs["x"], f32)
    Wq, bq = np.asarray(inputs["Wq"], f32), np.asarray(inputs["bq"], f32)
    Wk, bk = np.asarray(inputs["Wk"], f32), np.asarray(inputs["bk"], f32)
    Wv, bv = np.asarray(inputs["Wv"], f32), np.asarray(inputs["bv"], f32)
    Wo, bo = np.asarray(inputs["Wo"], f32), np.asarray(inputs["bo"], f32)
    rel_bias = np.asarray(inputs["rel_bias"], f32)
    gamma, beta = np.asarray(inputs["gamma"], f32), np.asarray(inputs["beta"], f32)

    xf = x.reshape(N, D)
    xT_bf = np.ascontiguousarray(xf.T).astype(BF)

    # host prep: transposed+masked rel_bias in bf16 (one pass per head)
    kq = np.arange(S)
    causal_mask = kq[:, None] > kq[None, :]     # True where key > query (masked)
    relT_bf = np.empty((H, S, S), dtype=BF)
    for h in range(H):
        m = rel_bias[h].T.copy()
        m[causal_mask] = NEG
        relT_bf[h] = m.astype(BF)

    scale = 1.0 / np.sqrt(np.float32(HD))
    in1 = []
    for c in range(NCORES):
        sl = slice(c * 128, (c + 1) * 128)
        in1.append({
            "xT": xT_bf,
            "wqT": np.ascontiguousarray((Wq[sl] * scale).T).astype(BF),
            "wkT": np.ascontiguousarray(Wk[sl].T).astype(BF),
            "wvT": np.ascontiguousarray(Wv[sl].T).astype(BF),
            "bq": (bq[sl] * scale).reshape(128, 1).astype(f32),
            "bk": bk[sl].reshape(128, 1).astype(f32),
            "relT": relT_bf[HPC * c: HPC * (c + 1)],
        })

    if "nc1" not in _CACHE:
        _CACHE["nc1"] = _build_phase1()
    res1 = _run(_CACHE["nc1"], in1, "phase1")
    attT_full = np.concatenate([res1[c]["attT"] for c in range(NCORES)], axis=0)

    woT_bf = np.ascontiguousarray(Wo.T).astype(BF)
    b2 = (bo + bv @ Wo.T).reshape(1, D).astype(f32)
    g2 = gamma.reshape(1, D).astype(f32)
    be2 = beta.reshape(1, D).astype(f32)
    in2 = []
    for c in range(NCORES):
        rs = slice(c * ROWS, (c + 1) * ROWS)
        in2.append({
            "aT": np.ascontiguousarray(attT_full[:, rs]),
            "woT": woT_bf,
            "xr": np.ascontiguousarray(xf[rs]),
            "b2": b2,
            "gam": g2,
            "bet": be2,
        })

    if "nc2" not in _CACHE:
        _CACHE["nc2"] = _build_phase2()
    res2 = _run(_CACHE["nc2"], in2, "phase2")
    y = np.concatenate([res2[c]["yout"] for c in range(NCORES)], axis=0)
    return y.reshape(B, S, D).astype(f32)


# revision 37
# speedup vs baseline: 108.0992x; 108.0992x over previous
# Trainium2 Bass kernel for CausalTemporalAttention
#   y = LayerNorm(x + Wo(softmax(QK^T/sqrt(hd) + causal + rel_bias) V))
#
# Strategy (8 NeuronCores):
#   Phase 1 (tensor-parallel over heads, 2 heads/core):
#     Each core computes Q^T,K^T (transposed via host-pretransposed x^T),
#     V (natural layout with an appended ones-column so the PV matmul also
#     produces softmax denominators), then causal attention with the
#     rel_bias streamed in transposed bf16 form with the causal mask baked
#     in on the host as -60000 (exp -> exact 0).  Softmax is computed
#     without max-subtraction (scores are bounded ~|4| for this problem).
#     Output: normalized attended^T (128 channels x 4096 rows) per core.
#   Host: concatenates the 8 channel-blocks (pure gather/reshard).
#   Phase 2 (sequence-parallel, 512 rows/core):
#     out-projection + residual + LayerNorm on each core's row block.
#     bv/bo are folded into a single bias vector on the host
#     (b2 = bo + bv @ Wo.T, parameter algebra only).
#
# All matmul math runs in bf16 with f32 PSUM accumulation (validated:
# absmax err ~5e-3 = 1e-3 of output scale vs the f32 reference).

import os
from contextlib import ExitStack

import ml_dtypes
import numpy as np

B, S, D, H, HD = 2, 2048, 1024, 16, 64
N = B * S            # 4096 flattened rows
NCORES = 8
HPC = H // NCORES    # 2 heads per core
ROWS = N // NCORES   # 512 rows per core in phase 2
QC = 512             # query-chunk width (phase 1)
KT = 128             # key-tile height (phase 1)
NEG = -60000.0       # causal mask additive value (exp -> 0 exactly in f32)
EPS = 1e-5

BF = ml_dtypes.bfloat16
REL_VIA_PE = True   # rel_bias into PSUM via identity-matmul (frees DVE)

_CACHE: dict = {}
LAST_PROFILE: list = []   # [(phase, exec_time_ns)], filled when BASS_TRACE=1


def _mybir():
    from concourse import mybir
    return mybir


def _build_phase1(loop_reps=False, bench_seed=False):
    import concourse.tile as tile
    from concourse import bacc
    mybir = _mybir()
    bf, f32 = mybir.dt.bfloat16, mybir.dt.float32
    Ident = mybir.ActivationFunctionType.Identity
    Exp = mybir.ActivationFunctionType.Exp

    nc = bacc.Bacc(
        "TRN2",
        target_bir_lowering=False,
        debug=False,
        enable_asserts=False,
        num_devices=NCORES,
    )
    xT = nc.dram_tensor("xT", (D, N), bf, kind="ExternalInput").ap()
    wqT = nc.dram_tensor("wqT", (D, 128), bf, kind="ExternalInput").ap()
    wkT = nc.dram_tensor("wkT", (D, 128), bf, kind="ExternalInput").ap()
    wvT = nc.dram_tensor("wvT", (D, 128), bf, kind="ExternalInput").ap()
    bq = nc.dram_tensor("bq", (128, 1), f32, kind="ExternalInput").ap()
    bk = nc.dram_tensor("bk", (128, 1), f32, kind="ExternalInput").ap()
    relT = nc.dram_tensor("relT", (HPC, S, S), bf, kind="ExternalInput").ap()
    if loop_reps:
        nreps = nc.dram_tensor("nreps", (1, 1), mybir.dt.int32, kind="ExternalInput").ap()
    if bench_seed:
        seed = nc.dram_tensor("seed", (1, 1), f32, kind="ExternalInput").ap()
        sout = nc.dram_tensor("sout", (1, 1), f32, kind="ExternalOutput").ap()
    attT = nc.dram_tensor("attT", (128, N), bf, kind="ExternalOutput").ap()

    with tile.TileContext(nc) as tc, ExitStack() as ctx:
        singles = ctx.enter_context(tc.tile_pool(name="singles", bufs=1))
        relp = ctx.enter_context(tc.tile_pool(name="relp", bufs=2))
        if REL_VIA_PE:
            ptp = ctx.enter_context(tc.tile_pool(name="ptp", bufs=10))
        else:
            stgp = ctx.enter_context(tc.tile_pool(name="stgp", bufs=4))
            ptp = ctx.enter_context(tc.tile_pool(name="ptp", bufs=6))
        smallp = ctx.enter_context(tc.tile_pool(name="smallp", bufs=2))
        attp = ctx.enter_context(tc.tile_pool(name="attp", bufs=2))
        if REL_VIA_PE:
            psA = ctx.enter_context(tc.tile_pool(name="psA", bufs=3, space="PSUM"))
            psPV = ctx.enter_context(tc.tile_pool(name="psPV", bufs=1, space="PSUM"))
            psBC = ctx.enter_context(tc.tile_pool(name="psBC", bufs=1, space="PSUM"))
        else:
            psA = ctx.enter_context(tc.tile_pool(name="psA", bufs=4, space="PSUM"))
            psPV = ctx.enter_context(tc.tile_pool(name="psPV", bufs=2, space="PSUM"))
            psBC = ctx.enter_context(tc.tile_pool(name="psBC", bufs=2, space="PSUM"))

        rep_ctx = ExitStack()
        if loop_reps:
            rep_sb = singles.tile([1, 1], mybir.dt.int32)
            nc.sync.dma_start(out=rep_sb, in_=nreps)
            nv = nc.values_load(rep_sb[0:1, 0:1], min_val=0, max_val=100000)
            rep_ctx.enter_context(tc.For_i(0, nv, 1))
        if bench_seed:
            s_sb = singles.tile([1, 1], f32)
            nc.sync.dma_start(out=s_sb, in_=seed)
            nc.sync.dma_start(out=sout, in_=s_sb)

        # ---- load inputs (spread across DGE queues for parallel DMA) ----
        dmae = [nc.sync, nc.scalar, nc.gpsimd]
        xt = singles.tile([128, 8, N], bf)
        xTr = xT.rearrange("(c p) n -> c p n", p=128)
        for i in range(8):
            dmae[i % 3].dma_start(out=xt[:, i, :], in_=xTr[i])

        wq = singles.tile([128, 8, 128], bf)
        wk = singles.tile([128, 8, 128], bf)
        wv = singles.tile([128, 8, 128], bf)
        for j, (w_sb, w_d) in enumerate(((wk, wkT), (wq, wqT), (wv, wvT))):
            w_r = w_d.rearrange("(c p) m -> c p m", p=128)
            nc.gpsimd.dma_start(out=w_sb[:, :, :], in_=w_r.rearrange("c p m -> p c m"))
        bq_sb = singles.tile([128, 1], f32)
        nc.gpsimd.dma_start(out=bq_sb, in_=bq)
        bk_sb = singles.tile([128, 1], f32)
        nc.gpsimd.dma_start(out=bk_sb, in_=bk)
        ones_sb = singles.tile([1, 64], f32)
        nc.vector.memset(ones_sb, 1.0)
        if REL_VIA_PE:
            from concourse.masks import make_identity
            ident = singles.tile([128, 128], bf)
            make_identity(nc, ident)

        # per-chunk tiles so attention can start before projections finish
        qt = [singles.tile([128, QC], bf, tag=f"qt{i}", name=f"qt{i}") for i in range(N // QC)]
        ktl = [singles.tile([128, QC], bf, tag=f"kt{i}", name=f"ktl{i}") for i in range(N // QC)]
        vt = [singles.tile([128, 130], bf, tag=f"vt{m}", name=f"vt{m}") for m in range(N // KT)]
        for m in range(N // KT):
            nc.vector.memset(vt[m][:, 64:65], 1.0)
            nc.vector.memset(vt[m][:, 129:130], 1.0)

        # ---- K/Q projections (transposed layout) ----
        for w_sb, b_sb, dst in ((wk, bk_sb, ktl), (wq, bq_sb, qt)):
            for ncol in range(N // QC):
                p = psA.tile([128, QC], f32, tag="ps")
                for i in range(8):
                    nc.tensor.matmul(
                        p,
                        lhsT=w_sb[:, i, :],
                        rhs=xt[:, i, ncol * QC:(ncol + 1) * QC],
                        start=(i == 0),
                        stop=(i == 7),
                    )
                nc.vector.tensor_scalar_add(out=dst[ncol], in0=p,
                                            scalar1=b_sb[:, 0:1])

        # ---- V projection (natural layout) ----
        for m in range(N // KT):
            p = psA.tile([128, 128], f32, tag="ps")
            for i in range(8):
                nc.tensor.matmul(
                    p,
                    lhsT=xt[:, i, m * KT:(m + 1) * KT],
                    rhs=wv[:, i, :],
                    start=(i == 0),
                    stop=(i == 7),
                )
            nc.vector.tensor_copy(out=vt[m][:, 0:64], in_=p[:, 0:64])
            nc.vector.tensor_copy(out=vt[m][:, 65:129], in_=p[:, 64:128])

        # ---- attention ----
        relr = relT.rearrange("h (t p) q -> h p t q", p=128)
        nqc = S // QC             # 4 query chunks per batch
        for h in range(HPC):
            for qc in range(nqc):
                nkt = (qc + 1) * (QC // KT)   # causal: k-tiles needed
                rel = relp.tile([128, S // KT, QC], bf, tag="rel")
                nc.sync.dma_start(
                    out=rel[:, 0:nkt, :],
                    in_=relr[h][:, 0:nkt, qc * QC:(qc + 1) * QC],
                )
                for b in range(B):
                    qoff = b * S + qc * QC
                    ptg = []
                    if REL_VIA_PE:
                        for g in range(nkt // 2):
                            sp2 = psA.tile([128, 2, QC], f32, tag="ps",
                                           name=f"sp{h}{qc}{b}{g}")
                            pts = ptp.tile([128, 2, QC], bf, tag="pt",
                                           name=f"pts{h}{qc}{b}{g}")
                            ptg.append(pts)
                            for j in range(2):
                                kt = 2 * g + j
                                nc.tensor.matmul(sp2[:, j, :], lhsT=ident,
                                                 rhs=rel[:, kt, :],
                                                 start=True, stop=False)
                                nc.tensor.matmul(
                                    sp2[:, j, :],
                                    lhsT=ktl[b * 4 + kt // 4][64 * h:64 * (h + 1),
                                                              (kt % 4) * KT:(kt % 4 + 1) * KT],
                                    rhs=qt[b * 4 + qc][64 * h:64 * (h + 1), :],
                                    start=False,
                                    stop=True,
                                )
                            nc.scalar.activation(out=pts, in_=sp2, func=Exp)
                        kgrp = 2
                    else:
                        for g in range(nkt // 4):
                            stg = stgp.tile([128, 4, QC], bf, tag="stg",
                                            name=f"stg{h}{qc}{b}{g}")
                            pts = ptp.tile([128, 4, QC], bf, tag="pt",
                                           name=f"pts{h}{qc}{b}{g}")
                            ptg.append(pts)
                            for kt in range(4 * g, 4 * g + 4):
                                sp = psA.tile([128, QC], f32, tag="ps")
                                nc.tensor.matmul(
                                    sp,
                                    lhsT=ktl[b * 4 + kt // 4][64 * h:64 * (h + 1),
                                                              (kt % 4) * KT:(kt % 4 + 1) * KT],
                                    rhs=qt[b * 4 + qc][64 * h:64 * (h + 1), :],
                                    start=True,
                                    stop=True,
                                )
                                nc.vector.tensor_add(out=stg[:, kt % 4, :], in0=sp,
                                                     in1=rel[:, kt, :])
                            nc.scalar.activation(out=pts, in_=stg, func=Exp)
                        kgrp = 4
                    pv = psPV.tile([128, QC], f32, tag="pv")
                    for kt in range(nkt):
                        nc.tensor.matmul(
                            pv[0:65, :],
                            lhsT=vt[b * (S // KT) + kt][:, h * 65:(h + 1) * 65],
                            rhs=ptg[kt // kgrp][:, kt % kgrp, :],
                            start=(kt == 0),
                            stop=(kt == nkt - 1),
                        )
                    rec = smallp.tile([1, QC], f32, tag="rec")
                    nc.vector.reciprocal(out=rec, in_=pv[64:65, :])
                    bc = psBC.tile([64, QC], f32, tag="bc")
                    nc.tensor.matmul(bc, lhsT=ones_sb, rhs=rec, start=True, stop=True)
                    bcs = smallp.tile([64, QC], f32, tag="bcs")
                    nc.vector.tensor_copy(out=bcs, in_=bc)
                    att = attp.tile([64, QC], bf, tag="att")
                    nc.vector.tensor_mul(out=att, in0=pv[0:64, :], in1=bcs)
                    nc.sync.dma_start(
                        out=attT[64 * h:64 * (h + 1), qoff:qoff + QC], in_=att
                    )
        rep_ctx.close()
    nc.compile()
    return nc


def _build_phase2(loop_reps=False, bench_seed=False):
    import concourse.tile as tile
    from concourse import bacc
    mybir = _mybir()
    bf, f32 = mybir.dt.bfloat16, mybir.dt.float32
    Sqrt = mybir.ActivationFunctionType.Sqrt

    nc = bacc.Bacc(
        "TRN2",
        target_bir_lowering=False,
        debug=False,
        enable_asserts=False,
        num_devices=NCORES,
    )
    aT = nc.dram_tensor("aT", (D, ROWS), bf, kind="ExternalInput").ap()
    woT = nc.dram_tensor("woT", (D, D), bf, kind="ExternalInput").ap()
    xr = nc.dram_tensor("xr", (ROWS, D), f32, kind="ExternalInput").ap()
    b2 = nc.dram_tensor("b2", (1, D), f32, kind="ExternalInput").ap()
    gam = nc.dram_tensor("gam", (1, D), f32, kind="ExternalInput").ap()
    bet = nc.dram_tensor("bet", (1, D), f32, kind="ExternalInput").ap()
    if loop_reps:
        nreps = nc.dram_tensor("nreps", (1, 1), mybir.dt.int32, kind="ExternalInput").ap()
    if bench_seed:
        seed = nc.dram_tensor("seed", (1, 1), f32, kind="ExternalInput").ap()
        sout = nc.dram_tensor("sout", (1, 1), f32, kind="ExternalOutput").ap()
    yout = nc.dram_tensor("yout", (ROWS, D), f32, kind="ExternalOutput").ap()

    import concourse.bass as bass

    def bcast(ap_1d):
        # (1, D) dram -> [0-stride over 128 partitions, D] for broadcast DMA
        return bass.AP(tensor=ap_1d.tensor, offset=ap_1d.offset,
                       ap=[[0, 128]] + list(ap_1d.ap[1:]))

    with tile.TileContext(nc) as tc, ExitStack() as ctx:
        singles = ctx.enter_context(tc.tile_pool(name="singles", bufs=1))
        work = ctx.enter_context(tc.tile_pool(name="work", bufs=3))
        ps = ctx.enter_context(tc.tile_pool(name="ps", bufs=4, space="PSUM"))

        rep_ctx = ExitStack()
        if loop_reps:
            rep_sb = singles.tile([1, 1], mybir.dt.int32)
            nc.sync.dma_start(out=rep_sb, in_=nreps)
            nv = nc.values_load(rep_sb[0:1, 0:1], min_val=0, max_val=100000)
            rep_ctx.enter_context(tc.For_i(0, nv, 1))
        if bench_seed:
            s_sb = singles.tile([1, 1], f32)
            nc.sync.dma_start(out=s_sb, in_=seed)
            nc.sync.dma_start(out=sout, in_=s_sb)

        dmae = [nc.sync, nc.scalar, nc.gpsimd]
        at = [singles.tile([128, ROWS], bf, tag=f"at{i}", name=f"at{i}")
              for i in range(8)]
        a_r = aT.rearrange("(c p) n -> c p n", p=128)
        for i in range(8):
            dmae[i % 3].dma_start(out=at[i], in_=a_r[i])
        wo = [singles.tile([128, D], bf, tag=f"wo{i}", name=f"wo{i}")
              for i in range(8)]
        w_r = woT.rearrange("(c p) n -> c p n", p=128)
        for i in range(8):
            dmae[(i + 1) % 3].dma_start(out=wo[i], in_=w_r[i])
        b2_sb = singles.tile([1, D], f32)
        nc.sync.dma_start(out=b2_sb, in_=b2)
        ones1 = singles.tile([1, 128], f32)
        nc.vector.memset(ones1, 1.0)
        g_sb = singles.tile([128, D], f32)
        nc.sync.dma_start(out=g_sb, in_=bcast(gam))
        be_sb = singles.tile([128, D], f32)
        nc.sync.dma_start(out=be_sb, in_=bcast(bet))
        eps_sb = singles.tile([128, 1], f32)
        nc.vector.memset(eps_sb, EPS)

        for m in range(ROWS // 128):
            xr_sb = work.tile([128, D], f32, tag="xr")
            dmae[m % 3].dma_start(out=xr_sb, in_=xr[m * 128:(m + 1) * 128, :])
            y = work.tile([128, D], f32, tag="y")
            for ncc in range(D // 512):
                p = ps.tile([128, 512], f32, tag="p")
                for i in range(8):
                    nc.tensor.matmul(
                        p,
                        lhsT=at[i][:, m * 128:(m + 1) * 128],
                        rhs=wo[i][:, ncc * 512:(ncc + 1) * 512],
                        start=(i == 0),
                        stop=False,
                    )
                # += b2 (ones[1,128].T @ b2[1,512] broadcasts b2 to all rows)
                nc.tensor.matmul(
                    p,
                    lhsT=ones1,
                    rhs=b2_sb[:, ncc * 512:(ncc + 1) * 512],
                    start=False,
                    stop=True,
                )
                nc.vector.tensor_add(
                    out=y[:, ncc * 512:(ncc + 1) * 512],
                    in0=p,
                    in1=xr_sb[:, ncc * 512:(ncc + 1) * 512],
                )
            stats = work.tile([128, 2, 6], f32, tag="st")
            nc.vector.bn_stats(out=stats[:, 0, :], in_=y[:, 0:512])
            nc.vector.bn_stats(out=stats[:, 1, :], in_=y[:, 512:1024])
            mv = work.tile([128, 2], f32, tag="mv")
            nc.vector.bn_aggr(out=mv, in_=stats)
            rstd = work.tile([128, 1], f32, tag="rs")
            nc.scalar.activation(out=rstd, in_=mv[:, 1:2], func=Sqrt,
                                 bias=eps_sb[:, 0:1], scale=1.0)
            nc.vector.reciprocal(out=rstd, in_=rstd)
            nc.vector.tensor_scalar(
                out=y, in0=y,
                scalar1=mv[:, 0:1], scalar2=rstd,
                op0=mybir.AluOpType.subtract, op1=mybir.AluOpType.mult,
            )
            nc.vector.tensor_mul(out=y, in0=y, in1=g_sb)
            nc.vector.tensor_add(out=y, in0=y, in1=be_sb)
            dmae[(m + 1) % 3].dma_start(out=yout[m * 128:(m + 1) * 128, :], in_=y)
        rep_ctx.close()
    nc.compile()
    return nc


def _run(nc, in_maps, phase):
    from concourse.bass_utils import run_bass_kernel_spmd
    trace = bool(os.environ.get("BASS_TRACE"))
    res = run_bass_kernel_spmd(nc, in_maps, list(range(NCORES)), trace=trace)
    if trace:
        LAST_PROFILE.append((phase, res.exec_time_ns))
    return res.results


def kernel(**inputs):
    f32 = np.float32
    x = np.asarray(inputs["x"], f32)
    Wq, bq = np.asarray(inputs["Wq"], f32), np.asarray(inputs["bq"], f32)
    Wk, bk = np.asarray(inputs["Wk"], f32), np.asarray(inputs["bk"], f32)
    Wv, bv = np.asarray(inputs["Wv"], f32), np.asarray(inputs["bv"], f32)
    Wo, bo = np.asarray(inputs["Wo"], f32), np.asarray(inputs["bo"], f32)
    rel_bias = np.asarray(inputs["rel_bias"], f32)
    gamma, beta = np.asarray(inputs["gamma"], f32), np.asarray(inputs["beta"], f32)

    xf = x.reshape(N, D)
    xT_bf = np.ascontiguousarray(xf.T).astype(BF)

    # host prep: transposed+masked rel_bias in bf16 (one pass per head)
    kq = np.arange(S)
    causal_mask = kq[:, None] > kq[None, :]     # True where key > query (masked)
    relT_bf = np.empty((H, S, S), dtype=BF)
    for h in range(H):
        m = rel_bias[h].T.copy()
        m[causal_mask] = NEG
        relT_bf[h] = m.astype(BF)

    scale = 1.0 / np.sqrt(np.float32(HD))
    in1 = []
    for c in range(NCORES):
        sl = slice(c * 128, (c + 1) * 128)
        in1.append({
            "xT": xT_bf,
            "wqT": np.ascontiguousarray((Wq[sl] * scale).T).astype(BF),
            "wkT": np.ascontiguousarray(Wk[sl].T).astype(BF),
            "wvT": np.ascontiguousarray(Wv[sl].T).astype(BF),
            "bq": (bq[sl] * scale).reshape(128, 1).astype(f32),
            "bk": bk[sl].reshape(128, 1).astype(f32),
            "relT": relT_bf[HPC * c: HPC * (c + 1)],
        })

    if "nc1" not in _CACHE:
        _CACHE["nc1"] = _build_phase1()
    res1 = _run(_CACHE["nc1"], in1, "phase1")
    attT_full = np.concatenate([res1[c]["attT"] for c in range(NCORES)], axis=0)

    woT_bf = np.ascontiguousarray(Wo.T).astype(BF)
    b2 = (bo + bv @ Wo.T).reshape(1, D).astype(f32)
    g2 = gamma.reshape(1, D).astype(f32)
    be2 = beta.reshape(1, D).astype(f32)
    in2 = []
    for c in range(NCORES):
        rs = slice(c * ROWS, (c + 1) * ROWS)
        in2.append({
            "aT": np.ascontiguousarray(attT_full[:, rs]),
            "woT": woT_bf,
            "xr": np.ascontiguousarray(xf[rs]),
            "b2": b2,
            "gam": g2,
            "bet": be2,
        })

    if "nc2" not in _CACHE:
        _CACHE["nc2"] = _build_phase2()
    res2 = _run(_CACHE["nc2"], in2, "phase2")
    y = np.concatenate([res2[c]["yout"] for c in range(NCORES)], axis=0)
    return y.reshape(B, S, D).astype(f32)
